# revision 32
# baseline (speedup 1.0000x reference)
"""Multi-head attention (B=4, S=2048, H=1024, NH=16) on 8 trn2 NeuronCores.

Sharding: token-parallel, no collectives. Core c handles batch b=c//2,
query half h=c%2 (1024 query tokens), with the full 2048-key K/V of its
batch (K/V projection duplicated within each core pair).

Per-core pipeline (bf16 matmul inputs, fp32 PSUM accumulation):
  A) Q projection -> per-head zero-padded feature-major slabs Qz[h]
     spilled to internal DRAM (head features in its 64-row half, zeros
     elsewhere) so the QK^T contraction runs at the full K=128 rate
     (K=64 matmuls measured ~2x slower).  K projection -> feature-major
     slabs spilled to DRAM.  Q/K biases are added during the DVE PSUM
     evacuation.  V projection -> SBUF-resident token-major Vt
     [tok, 16*65] with a per-head ones column (the PV matmul then also
     produces the softmax denominator), emitted in 4-head column
     quarters so PV can start early; V bias via a K=1 ones-row matmul.
     The 1/8 attention scale is folded into Wq on the host.
  B) Attention emitted interleaved with the K/V projections so the
     ScalarE exp stream starts within ~40us and runs continuously:
     per head, scoresT[k,q] = Kt-chunk^T x Qz_h (K=128), exp on ScalarE
     (attention-mask as per-partition bias), PV ctx[q, 65] accumulated
     over 16 key chunks with two interleaved query-tile chains,
     normalized by the ones-column denominator during evacuation.
  C) PE-transpose ctx to feature-major, output projection token-major
     (both 512-col chains share the stationary operand), bias via
     ones-row matmul, DMA out [1024 tok, 1024] fp32.
"""

import numpy as np
import ml_dtypes

import concourse.tile as tile
from concourse import bacc, mybir
from concourse.bass_utils import run_bass_kernel_spmd
from concourse.masks import make_identity

B, S, H, NH, HDIM = 4, 2048, 1024, 16, 64
NCORES = 8
TOK = 1024            # query tokens per core
KTOK = 2048           # key tokens per core
IC = H // 128         # 8 feature chunks of 128
KC = KTOK // 128      # 16 key chunks of 128
QT = TOK // 128       # 8 query tiles of 128
VW = NH * (HDIM + 1)  # 1040: V columns incl. per-head ones column
VQ = VW // 4          # 260: V column quarter = 4 heads
BF = mybir.dt.bfloat16
F32 = mybir.dt.float32
E_BUFS = 22

_CACHE = {}


def _emit(nc, tc, io):
    Exp = mybir.ActivationFunctionType.Exp

    persist = tc.alloc_tile_pool(name="persist", bufs=1)
    psum = tc.alloc_tile_pool(name="psum", bufs=2, space="PSUM")
    attnp = tc.alloc_tile_pool(name="attnp", bufs=1)

    kt_store = nc.dram_tensor("kt_store", [IC, 128, KTOK], BF).ap()
    qz_store = nc.dram_tensor("qz_store", [NH, 64, TOK], BF).ap()

    # ---- persistent tiles ----
    ones = persist.tile([1, 512], BF, name="ones", tag="ones")
    nc.vector.memset(ones[:], 1.0)
    ident = persist.tile([128, 128], BF, name="ident", tag="ident")
    make_identity(nc, ident[:])
    mask_sb = persist.tile([128, KC], F32, name="mask_sb", tag="mask_sb")
    nc.sync.dma_start(mask_sb[:], io["maskcol"][:])
    bqc = persist.tile([128, IC], F32, name="bqc", tag="bqc")
    nc.sync.dma_start(bqc[:], io["bqcol"][:])
    bkc = persist.tile([128, IC], F32, name="bkc", tag="bkc")
    nc.sync.dma_start(bkc[:], io["bkcol"][:])



    Vt = [persist.tile([128, VW], BF, name=f"Vt{i}", tag=f"Vt{i}") for i in range(KC)]
    ctx = [persist.tile([128, H], BF, name=f"ctx{i}", tag=f"ctx{i}") for i in range(QT)]

    # ---- Q projection: og-major, zero-padded per-head slabs to DRAM ----
    def emit_q_proj():
        with tc.tile_pool(name="q_pool", bufs=1) as ap:
            x_s, w_s = [], []
            for i in range(IC):
                x = ap.tile([128, TOK], BF, name=f"q_x{i}", tag=f"qx{i}")
                nc.sync.dma_start(x[:, 0:512],
                                  io["qT"][i * 128:(i + 1) * 128, 0:512])
                nc.sync.dma_start(x[:, 512:1024],
                                  io["qT"][i * 128:(i + 1) * 128, 512:1024])
                x_s.append(x)
                w = ap.tile([128, H], BF, name=f"q_w{i}", tag=f"qw{i}")
                nc.sync.dma_start(w[:, 0:512],
                                  io["wqT"][i * 128:(i + 1) * 128, 0:512])
                w_s.append(w)
            for i in range(IC):
                nc.sync.dma_start(w_s[i][:, 512:1024],
                                  io["wqT"][i * 128:(i + 1) * 128, 512:1024])
            for og in range(IC):
                pa = psum.tile([128, 512], F32, name="ps_qa", tag="proj")
                pb = psum.tile([128, 512], F32, name="ps_qb", tag="proj")
                for i in range(IC):
                    w = w_s[i][:, og * 128:(og + 1) * 128]
                    nc.tensor.matmul(pa[:], w, x_s[i][:, 0:512],
                                     start=(i == 0), stop=(i == IC - 1))
                    nc.tensor.matmul(pb[:], w, x_s[i][:, 512:1024],
                                     start=(i == 0), stop=(i == IC - 1))
                for ps, tg in ((pa, 0), (pb, 1)):
                    cs = slice(tg * 512, (tg + 1) * 512)
                    se = ap.tile([128, 512], BF, name="q_se", tag="qse", bufs=4)
                    nc.vector.tensor_scalar_add(se[:], ps[:],
                                                bqc[:, og:og + 1])
                    nc.sync.dma_start(qz_store[2 * og][:, cs], se[0:64, :])
                    nc.sync.dma_start(qz_store[2 * og + 1][:, cs],
                                      se[64:128, :])

    # ---- K projection: feature-major slabs to DRAM, og granular ----
    k_pool = {}
    def open_k_pool():
        ap = tc.alloc_tile_pool(name="k_pool", bufs=1)
        k_pool["pool"] = ap
        k_pool["w"] = []
        k_pool["x"] = []
        for i in range(IC):
            w = ap.tile([128, H], BF, name=f"k_w{i}", tag=f"kw{i}", bufs=1)
            nc.sync.dma_start(w[:, 0:512],
                              io["wkT"][i * 128:(i + 1) * 128, 0:512])
            k_pool["w"].append(w)
            x = ap.tile([128, KTOK], BF, name=f"k_x{i}", tag=f"kx{i}", bufs=1)
            nc.sync.dma_start(x[:, 0:TOK],
                              io["kT"][i * 128:(i + 1) * 128, 0:TOK])
            nc.sync.dma_start(x[:, TOK:KTOK],
                              io["kT"][i * 128:(i + 1) * 128, TOK:KTOK])
            k_pool["x"].append(x)
        for i in range(IC):
            nc.sync.dma_start(k_pool["w"][i][:, 512:1024],
                              io["wkT"][i * 128:(i + 1) * 128, 512:1024])
        k_pool["kt0"] = ap.tile([128, KTOK], BF, name="kt0", tag="kt0", bufs=1)

    def emit_k_og(og):
        ap, w_s, x_s = k_pool["pool"], k_pool["w"], k_pool["x"]
        for tp in range(2):
            pa = psum.tile([128, 512], F32, name="ps_ka", tag="proj")
            pb = psum.tile([128, 512], F32, name="ps_kb", tag="proj")
            for i in range(IC):
                w = w_s[i][:, og * 128:(og + 1) * 128]
                nc.tensor.matmul(pa[:], w, x_s[i][:, tp * 1024:tp * 1024 + 512],
                                 start=(i == 0), stop=(i == IC - 1))
                nc.tensor.matmul(pb[:], w, x_s[i][:, tp * 1024 + 512:tp * 1024 + 1024],
                                 start=(i == 0), stop=(i == IC - 1))
            for ps, tg in ((pa, 0), (pb, 1)):
                col = tp * 1024 + tg * 512
                if og == 0:
                    # heads 0/1 read Kt og0 straight from SBUF: skip the
                    # DRAM round-trip on the critical path to the first exp
                    nc.vector.tensor_scalar_add(
                        k_pool["kt0"][:, col:col + 512], ps[:],
                        bkc[:, og:og + 1])
                else:
                    se = ap.tile([128, 512], BF, name="k_se", tag="kse", bufs=2)
                    nc.vector.tensor_scalar_add(se[:], ps[:], bkc[:, og:og + 1])
                    nc.sync.dma_start(kt_store[og][:, col:col + 512], se[:])

    # ---- V projection: token-major into SBUF Vt, quarter x th granular ----
    v_pool = {}
    def open_v_pool():
        ap = tc.alloc_tile_pool(name="v_pool", bufs=1)
        v_pool["pool"] = ap
        v_pool["w"] = []
        for i in range(IC):
            w = ap.tile([128, VW], BF, name=f"v_w{i}", tag=f"vw{i}", bufs=1)
            nc.sync.dma_start(w[:, 0:VQ],
                              io["wvT"][i * 128:(i + 1) * 128, 0:VQ])
            nc.sync.dma_start(w[:, VQ:VW],
                              io["wvT"][i * 128:(i + 1) * 128, VQ:VW])
            v_pool["w"].append(w)
        bv_s = ap.tile([1, VW], BF, name="v_b", tag="vb", bufs=1)
        nc.sync.dma_start(bv_s[:], io["bv"][:])
        v_pool["b"] = bv_s

    def emit_v_quarter(vq):
        ap, wv_s, bv_s = v_pool["pool"], v_pool["w"], v_pool["b"]
        o0 = vq * VQ
        for th in range(2):
            v_s = []
            for i in range(IC):
                x = ap.tile([128, TOK], BF, name=f"v_x{i}", tag=f"vx{i}", bufs=1)
                nc.sync.dma_start(x[:], io["vT"][i * 128:(i + 1) * 128,
                                                 th * TOK:(th + 1) * TOK])
                v_s.append(x)
            for tp in range(QT // 2):
                ka, kb = th * QT + 2 * tp, th * QT + 2 * tp + 1
                pa = psum.tile([128, VQ], F32, name="ps_va", tag="proj")
                pb = psum.tile([128, VQ], F32, name="ps_vb", tag="proj")
                for i in range(IC):
                    rhs = wv_s[i][:, o0:o0 + VQ]
                    nc.tensor.matmul(pa[:], v_s[i][:, (2 * tp) * 128:(2 * tp + 1) * 128],
                                     rhs, start=(i == 0), stop=False)
                    nc.tensor.matmul(pb[:], v_s[i][:, (2 * tp + 1) * 128:(2 * tp + 2) * 128],
                                     rhs, start=(i == 0), stop=False)
                nc.tensor.matmul(pa[:], ones[0:1, 0:128], bv_s[0:1, o0:o0 + VQ],
                                 start=False, stop=True)
                nc.tensor.matmul(pb[:], ones[0:1, 0:128], bv_s[0:1, o0:o0 + VQ],
                                 start=False, stop=True)
                nc.vector.tensor_copy(Vt[ka][:, o0:o0 + VQ], pa[:])
                nc.vector.tensor_copy(Vt[kb][:, o0:o0 + VQ], pb[:])

    # ---- attention ----
    kt_slabs = {}
    E_tiles = {}

    def emit_qkt_exp(h):
        hi = h // 2
        if hi == 0:
            kt = k_pool["kt0"]
        else:
            if h % 2 == 0:
                slab = attnp.tile([128, KTOK], BF, name=f"ktsl{hi}",
                                  tag="ktsl", bufs=2)
                nc.sync.dma_start(slab[:], kt_store[hi][:])
                kt_slabs[hi] = slab
            kt = kt_slabs[hi]
        hp = (h % 2) * 64
        qz = attnp.tile([128, TOK], BF, name=f"qzsl{h}", tag="qzsl", bufs=2)
        nc.vector.memset(qz[64 - hp:128 - hp, :], 0.0)
        nc.sync.dma_start(qz[hp:hp + 64, :], qz_store[h][:])
        E = []
        for kc in range(KC):
            ps_s = psum.tile([128, TOK], F32, name="ps_s", tag="scores")
            lhs = kt[:, kc * 128:(kc + 1) * 128]
            for qg in range(2):
                nc.tensor.matmul(
                    ps_s[:, qg * 512:(qg + 1) * 512], lhs,
                    qz[:, qg * 512:(qg + 1) * 512],
                    start=True, stop=True)
            e = attnp.tile([128, TOK], BF, name=f"E{kc}", tag="E", bufs=E_BUFS)
            nc.scalar.activation(e[:], ps_s[:], Exp,
                                 bias=mask_sb[:, kc:kc + 1], scale=1.0)
            E.append(e)
        E_tiles[h] = E

    def emit_pv(h, qps=None):
        E = E_tiles[h]
        if qps is None:
            qps = range(QT // 2)
        vs = slice(h * 65, h * 65 + 65)
        for qp in qps:
            qa, qb = 2 * qp, 2 * qp + 1
            pa = psum.tile([128, HDIM + 1], F32, name="ps_ca", tag="ctx")
            pb = psum.tile([128, HDIM + 1], F32, name="ps_cb", tag="ctx")
            for kc in range(KC):
                nc.tensor.matmul(pa[:], E[kc][:, qa * 128:(qa + 1) * 128],
                                 Vt[kc][:, vs],
                                 start=(kc == 0), stop=(kc == KC - 1))
                nc.tensor.matmul(pb[:], E[kc][:, qb * 128:(qb + 1) * 128],
                                 Vt[kc][:, vs],
                                 start=(kc == 0), stop=(kc == KC - 1))
            for ps, qt in ((pa, qa), (pb, qb)):
                rec = attnp.tile([128, 1], F32, name="rec", tag="rec", bufs=4)
                nc.vector.reciprocal(rec[:], ps[:, 64:65])
                nc.vector.tensor_scalar_mul(
                    ctx[qt][:, h * 64:(h + 1) * 64], ps[:, 0:64], rec[:])

    # ---- interleaved emission schedule ----
    H0, H1 = range(QT // 4), range(QT // 4, QT // 2)
    emit_q_proj()
    open_k_pool()
    emit_k_og(0); emit_qkt_exp(0)
    open_v_pool()
    emit_v_quarter(0)
    emit_qkt_exp(1); emit_pv(0)
    emit_k_og(1)
    emit_qkt_exp(2); emit_pv(1, H0); emit_qkt_exp(3); emit_pv(1, H1); emit_pv(2, H0)
    emit_k_og(2); emit_qkt_exp(4); emit_pv(2, H1)
    emit_pv(3, H0)
    emit_v_quarter(1)
    emit_qkt_exp(5); emit_pv(3, H1); emit_pv(4, H0)
    emit_k_og(3)
    emit_qkt_exp(6); emit_pv(4, H1); emit_pv(5, H0); emit_qkt_exp(7)
    emit_pv(5, H1); emit_pv(6, H0)
    emit_k_og(4); emit_qkt_exp(8); emit_pv(6, H1)
    emit_pv(7, H0)
    emit_v_quarter(2)
    emit_qkt_exp(9); emit_pv(7, H1); emit_pv(8, H0)
    emit_k_og(5)
    emit_qkt_exp(10); emit_pv(8, H1); emit_pv(9, H0); emit_qkt_exp(11)
    emit_pv(9, H1); emit_pv(10, H0)
    emit_k_og(6); emit_qkt_exp(12); emit_pv(10, H1)
    emit_pv(11, H0)
    emit_v_quarter(3)
    emit_qkt_exp(13); emit_pv(11, H1); emit_pv(12, H0)
    emit_k_og(7)
    emit_qkt_exp(14); emit_pv(12, H1); emit_pv(13, H0); emit_qkt_exp(15)
    emit_pv(13, H1); emit_pv(14); emit_pv(15)
    v_pool["pool"].release()
    k_pool["pool"].release()
    attnp.release()

    # ---- transpose ctx to feature-major; output projection ----
    with tc.tile_pool(name="o_pool", bufs=1) as cp:
        wo_s = []
        for i in range(IC):
            w = cp.tile([128, H], BF, name=f"o_w{i}", tag=f"ow{i}")
            nc.sync.dma_start(w[:], io["woT"][i * 128:(i + 1) * 128, :])
            wo_s.append(w)
        bo_s = cp.tile([1, H], BF, name="o_b", tag="ob")
        nc.sync.dma_start(bo_s[:], io["bo"][:])
        for tt in range(QT):
            osb = cp.tile([128, H], F32, name="osb", tag="osb", bufs=2)
            pa = psum.tile([128, 512], F32, name="ps_oa", tag="proj")
            pb = psum.tile([128, 512], F32, name="ps_ob", tag="proj")
            for i in range(IC):
                # transpose ctx[tt] chunk i on the fly and feed both chains
                ps_t = psum.tile([128, 128], BF, name="ps_t", tag="ctx")
                nc.tensor.transpose(
                    ps_t[:], ctx[tt][:, i * 128:(i + 1) * 128], ident[:])
                tr = cp.tile([128, 128], BF, name="tr", tag="tr", bufs=3)
                nc.vector.tensor_copy(tr[:], ps_t[:])
                nc.tensor.matmul(pa[:], tr[:], wo_s[i][:, 0:512],
                                 start=(i == 0), stop=False)
                nc.tensor.matmul(pb[:], tr[:], wo_s[i][:, 512:1024],
                                 start=(i == 0), stop=False)
            nc.tensor.matmul(pa[:], ones[0:1, 0:128], bo_s[0:1, 0:512],
                             start=False, stop=True)
            nc.tensor.matmul(pb[:], ones[0:1, 0:128], bo_s[0:1, 512:1024],
                             start=False, stop=True)
            nc.vector.tensor_copy(osb[:, 0:512], pa[:])
            nc.vector.tensor_copy(osb[:, 512:1024], pb[:])
            nc.sync.dma_start(io["out"][tt * 128:(tt + 1) * 128, :], osb[:])

    psum.release()
    persist.release()


def _build():
    nc = bacc.Bacc("TRN2", target_bir_lowering=False, debug=False,
                   num_devices=NCORES)
    io = {}
    def inp(name, shape, dtype=BF):
        io[name] = nc.dram_tensor(name, shape, dtype, kind="ExternalInput").ap()
    inp("qT", [H, TOK])
    inp("kT", [H, KTOK])
    inp("vT", [H, KTOK])
    inp("wqT", [H, H])
    inp("wkT", [H, H])
    inp("wvT", [H, VW])
    inp("woT", [H, H])
    inp("bv", [1, VW])
    inp("bo", [1, H])
    inp("bqcol", [128, IC], F32)
    inp("bkcol", [128, IC], F32)
    inp("maskcol", [128, KC], F32)
    io["out"] = nc.dram_tensor("out", [TOK, H], F32, kind="ExternalOutput").ap()

    with tile.TileContext(nc) as tc:
        _emit(nc, tc, io)
    nc.compile()
    return nc, io


def get_compiled():
    if "nc" not in _CACHE:
        _CACHE["nc"], _CACHE["io"] = _build()
    return _CACHE["nc"]


def make_in_maps(query, key_, value, attention_mask, Wq, bq, Wk, bk, Wv, bv,
                 Wo, bo):
    bf = ml_dtypes.bfloat16
    f32 = np.float32
    query = np.asarray(query, f32)
    key_ = np.asarray(key_, f32)
    value = np.asarray(value, f32)
    attention_mask = np.asarray(attention_mask, f32)
    Wq, bq = np.asarray(Wq, f32), np.asarray(bq, f32)
    Wk, bk = np.asarray(Wk, f32), np.asarray(bk, f32)
    Wv, bv = np.asarray(Wv, f32), np.asarray(bv, f32)
    Wo, bo = np.asarray(Wo, f32), np.asarray(bo, f32)

    scale = 1.0 / np.sqrt(np.float32(HDIM))
    wqT = np.ascontiguousarray((Wq * scale).T).astype(bf)
    wkT = np.ascontiguousarray(Wk.T).astype(bf)
    woT = np.ascontiguousarray(Wo.T).astype(bf)
    wvT = np.zeros((H, VW), f32)
    bv_ext = np.zeros((1, VW), f32)
    for h in range(NH):
        wvT[:, h * 65:h * 65 + 64] = Wv[h * 64:(h + 1) * 64, :].T
        bv_ext[0, h * 65:h * 65 + 64] = bv[h * 64:(h + 1) * 64]
        bv_ext[0, h * 65 + 64] = 1.0
    wvT = wvT.astype(bf)
    bv_ext = bv_ext.astype(bf)
    bo_s = bo.reshape(1, H).astype(bf)
    bqcol = np.ascontiguousarray((bq * scale).reshape(IC, 128).T).astype(f32)
    bkcol = np.ascontiguousarray(bk.reshape(IC, 128).T).astype(f32)

    in_maps = []
    for c in range(NCORES):
        b, half = divmod(c, 2)
        sl = slice(half * TOK, (half + 1) * TOK)
        qT = np.ascontiguousarray(query[b, sl, :].T).astype(bf)
        kT = np.ascontiguousarray(key_[b].T).astype(bf)
        vT = np.ascontiguousarray(value[b].T).astype(bf)
        maskcol = np.ascontiguousarray(
            ((1.0 - attention_mask[b]) * -10000.0).reshape(KC, 128).T
        ).astype(f32)
        in_maps.append({
            "qT": qT, "kT": kT, "vT": vT,
            "wqT": wqT, "wkT": wkT, "wvT": wvT, "woT": woT,
            "bv": bv_ext, "bo": bo_s,
            "bqcol": bqcol, "bkcol": bkcol,
            "maskcol": maskcol,
        })
    return in_maps


def kernel(query, key_, value, attention_mask, Wq, bq, Wk, bk, Wv, bv, Wo, bo,
           **run_kwargs):
    nc = get_compiled()
    in_maps = make_in_maps(query, key_, value, attention_mask, Wq, bq, Wk, bk,
                           Wv, bv, Wo, bo)
    res = run_bass_kernel_spmd(nc, in_maps, core_ids=list(range(NCORES)),
                               **run_kwargs)
    out = np.empty((B, S, H), np.float32)
    for c in range(NCORES):
        b, half = divmod(c, 2)
        out[b, half * TOK:(half + 1) * TOK, :] = res.results[c]["out"]
    if run_kwargs:
        kernel.last_results = res
    return out
